# revision 1
# baseline (speedup 1.0000x reference)
"""Trainium2 Bass kernel for nn_Block2DGRU (norm->dwconv3x3->bi-minGRU->norm->MLP).

Self-contained: host-side weight folding + sharding, device kernel via
Bass/Tile, SPMD over 8 NeuronCores (data-parallel over batch: 2 per core).

Device layout: everything [feature_on_partitions, time_on_free].  The minGRU
linear recurrence h_t = a_t*h_{t-1} + b_t runs natively on the DVE via
tensor_tensor_scan (per-partition scan along the free dim); GRU2 is the same
scan with reversed access patterns.
"""
import numpy as np

import concourse.bass as bass
import concourse.tile as tile
import concourse.mybir as mybir
from concourse.bass_utils import run_bass_kernel_spmd

F32 = mybir.dt.float32
F32R = mybir.dt.float32r
AF = mybir.ActivationFunctionType
ALU = mybir.AluOpType

# dims
NB = 56
L = NB * NB            # 3136
D = 384                # dim
DC = 3                 # dim chunks of 128
DI = 768               # gru inner
DIC = 6
MLP = 1536
MLPC = 12
B = 2                  # batch per core
NCORES = 8
NT = 392               # time block (= 7 image rows)
NBLK = L // NT         # 8
QT = 784               # scan quarter (= 2 blocks)
NQ = L // QT           # 4
EPS = 1e-5

MM_DT = F32R           # matmul dtype: F32 (exact, 4 cyc/row) or F32R (1 cyc/row)


# ---------------------------------------------------------------- wait fix
def _fix_multiwaits(nc):
    """This walrus accepts at most ONE sync wait per instruction; hoist
    extras into wait-only NoOps on the same engine (streams are in-order)."""
    n = 0
    cnt = [0]
    for f in nc.m.functions:
        for bb in f.blocks:
            out = []
            for inst in bb.instructions:
                si = inst.sync_info
                if si is not None and si.on_wait is not None and len(si.on_wait) > 1:
                    waits = list(si.on_wait)
                    for w in waits[:-1]:
                        cnt[0] += 1
                        nop = mybir.InstNoOp(
                            name=f"I-waitfix-{cnt[0]}",
                            sync_info=mybir.SyncInfo(on_wait=[w], on_update=[]),
                        )
                        nop.engine = inst.engine
                        out.append(nop)
                    inst.sync_info = mybir.SyncInfo(
                        on_wait=[waits[-1]], on_update=list(si.on_update or [])
                    )
                    n += 1
                out.append(inst)
            bb.instructions = out
    return n


# ---------------------------------------------------------------- builder
def _conv_tap_ranges(tap, slab):
    """valid out rows [r0, r1) within image and cols [c0, c1) for tap."""
    dr, dc = tap // 3 - 1, tap % 3 - 1
    rlo, rhi = max(0, -dr), min(NB - 1, NB - 1 - dr)
    r0 = max(7 * slab, rlo)
    r1 = min(7 * slab + 6, rhi)
    c0, c1 = max(0, -dc), min(NB - 1, NB - 1 - dc)
    return dr, dc, r0, r1 + 1, c0, c1 + 1


def build_kernel(mm_dt=MM_DT, reps=1):
    nc = bass.Bass("TRN2", target_bir_lowering=False, debug=False,
                   num_devices=NCORES)

    xT_d = nc.dram_tensor("xT", [B, D, L], F32, kind="ExternalInput").ap()
    whg1_d = nc.dram_tensor("whg1", [D, 2 * DI], F32, kind="ExternalInput").ap()
    whg2_d = nc.dram_tensor("whg2", [D, 2 * DI], F32, kind="ExternalInput").ap()
    wout1_d = nc.dram_tensor("wout1", [DI, D], F32, kind="ExternalInput").ap()
    wout2_d = nc.dram_tensor("wout2", [DI, D], F32, kind="ExternalInput").ap()
    p1_d = nc.dram_tensor("p1", [D, MLP], F32, kind="ExternalInput").ap()
    p2_d = nc.dram_tensor("p2", [MLP, D], F32, kind="ExternalInput").ap()
    diag_d = nc.dram_tensor("diag", [DC, 128, 9 * 128], F32, kind="ExternalInput").ap()
    dwb_d = nc.dram_tensor("dwb", [128, DC], F32, kind="ExternalInput").ap()
    p1b_d = nc.dram_tensor("p1b", [128, MLPC], F32, kind="ExternalInput").ap()
    p2b_d = nc.dram_tensor("p2b", [128, DC], F32, kind="ExternalInput").ap()
    out_d = nc.dram_tensor("outT", [B, D, L], F32, kind="ExternalOutput").ap()

    f32r = mm_dt == F32R

    from contextlib import ExitStack
    with tile.TileContext(nc) as tc, ExitStack() as ctx:
        big = ctx.enter_context(tc.tile_pool(name="big", bufs=1))
        wpool = ctx.enter_context(tc.tile_pool(name="wpool", bufs=1))
        work = ctx.enter_context(tc.tile_pool(name="work", bufs=2))
        psum = ctx.enter_context(tc.tile_pool(name="psum", bufs=1, space="PSUM"))
        psumb = ctx.enter_context(tc.tile_pool(name="psumb", bufs=2, space="PSUM"))

        # ---- persistent small constants
        ones_col_f = wpool.tile([128, 1], F32, tag="ones_col_f", name="ones_col_f")
        nc.vector.memset(ones_col_f[:], 1.0)
        ones1_f = wpool.tile([1, 128], F32, tag="ones1_f", name="ones1_f")
        nc.vector.memset(ones1_f[:], 1.0)
        if f32r:
            ones_col_r = wpool.tile([128, 1], F32R, tag="ones_col_r", name="ones_col_r")
            nc.vector.tensor_copy(ones_col_r[:], ones_col_f[:])
        else:
            ones_col_r = ones_col_f
        dwb_t = wpool.tile([128, DC], F32, tag="dwb", name="dwb")
        nc.sync.dma_start(dwb_t[:], dwb_d)
        p1b_t = wpool.tile([128, MLPC], F32, tag="p1b", name="p1b")
        nc.sync.dma_start(p1b_t[:], p1b_d)
        p2b_t = wpool.tile([128, DC], F32, tag="p2b", name="p2b")
        nc.sync.dma_start(p2b_t[:], p2b_d)
        eps_t = wpool.tile([1, 1], F32, tag="eps", name="eps")
        nc.vector.memset(eps_t[:], EPS)

        def load_w(src_ap, shape, tag, bufs=None):
            """DMA a weight slice; convert to mm_dt when needed."""
            if not f32r:
                t = wpool.tile(shape, F32, tag=tag, name=tag, bufs=bufs)
                nc.sync.dma_start(t[:], src_ap)
                return t
            t = wpool.tile(shape, F32R, tag=tag, name=tag, bufs=bufs)
            for c0 in range(0, shape[1], 768):
                cw = min(768, shape[1] - c0)
                st = wpool.tile([shape[0], 768], F32, tag="wstage",
                                name="wstage", bufs=2)
                nc.sync.dma_start(st[:, 0:cw], src_ap[:, c0:c0 + cw])
                nc.vector.tensor_copy(t[:, c0:c0 + cw], st[:, 0:cw])
            return t

        for rep in range(reps):
          for b in range(B):
            # ========================================== phase N1: layernorm1
            x_t = [big.tile([128, L], F32, tag=f"bufA{c}", name=f"bufA{c}")
                   for c in range(DC)]
            ob1 = range(NBLK)
            ob2 = range(NBLK - 1, -1, -1)
            for c in range(DC):
                for blk in ob1:
                    bsl = slice(blk * NT, (blk + 1) * NT)
                    nc.sync.dma_start(x_t[c][:, bsl],
                                      xT_d[b, c * 128:(c + 1) * 128, bsl])

            # stat rows: inv at partition 0, ninv at partition 32 (matmul rhs
            # needs base partition in {0,32,64}); numu/sd/m2/ve in strows.
            mmrows = work.tile([1, 2 * L], F32, tag="mmrows", name="mmrows",
                               bufs=1)
            numu_row = mmrows[0:1, 0:L]
            inv_row = mmrows[0:1, L:2 * L]
            strows = work.tile([97, NT], F32, tag="strows", name="strows",
                               bufs=1)

            def norm_stats(src_tiles, src_dt, sq_src_f32, order=None):
                ones_c = ones_col_r if src_dt == F32R else ones_col_f
                for blk in (order or range(NBLK)):
                    sl = slice(blk * NT, (blk + 1) * NT)
                    s_ps = psum.tile([1, NT], F32, tag="pA", name="s_ps")
                    q_ps = psum.tile([1, NT], F32, tag="pB", name="q_ps")
                    for c in range(DC):
                        sq = work.tile([128, NT], src_dt, tag="sq", name="sq",
                                       bufs=1)
                        nc.scalar.activation(sq[:], sq_src_f32(c, sl), AF.Square)
                        nc.tensor.matmul(s_ps[:], ones_c[:], src_tiles[c][:, sl],
                                         start=(c == 0), stop=(c == DC - 1))
                        nc.tensor.matmul(q_ps[:], ones_c[:], sq[:],
                                         start=(c == 0), stop=(c == DC - 1))
                    nc.scalar.activation(numu_row[:, sl], s_ps[:], AF.Copy,
                                         scale=-1.0 / D)
                    m2 = strows[64:65, 0:NT]
                    nc.scalar.activation(m2, s_ps[:], AF.Square, scale=1.0 / D)
                    ve = strows[96:97, 0:NT]
                    nc.vector.scalar_tensor_tensor(ve, q_ps[:], 1.0 / D,
                                                   m2, ALU.mult, ALU.subtract)
                    sd_blk = strows[32:33, 0:NT]
                    nc.scalar.activation(sd_blk, ve, AF.Sqrt, bias=eps_t[:])
                    pkb = work.tile([7, NB], F32, tag="pk", name="pkb", bufs=2)
                    nc.sync.dma_start(pkb[:], sd_blk)
                    ikb = work.tile([7, NB], F32, tag="ipk", name="ikb", bufs=2)
                    nc.vector.reciprocal(ikb[:], pkb[:])
                    nc.sync.dma_start(inv_row[:, sl], ikb[:])

            def norm_apply(dst_tiles, src_f32, order=None):
                """dst = (x + (-mu)) * inv, both rows broadcast via K=1 mms."""
                for blk in (order or range(NBLK)):
                    sl = slice(blk * NT, (blk + 1) * NT)
                    mb_ps = psum.tile([128, NT], F32,
                                      tag=("pC" if blk % 2 == 0 else "pA"),
                                      name="mb_ps")
                    nc.tensor.matmul(mb_ps[:], ones1_f[:], numu_row[:, sl],
                                     start=True, stop=True)
                    ib_ps = psum.tile([128, NT], F32,
                                      tag=("pD" if blk % 2 == 0 else "pB"),
                                      name="ib_ps")
                    nc.tensor.matmul(ib_ps[:], ones1_f[:], inv_row[:, sl],
                                     start=True, stop=True)
                    for c in range(DC):
                        t = work.tile([128, NT], F32, tag="t_ap", name="t_ap", bufs=3)
                        nc.vector.tensor_tensor(t[:], src_f32(c, sl), mb_ps[:],
                                                ALU.add)
                        nc.vector.tensor_tensor(dst_tiles[c][:, sl], t[:],
                                                ib_ps[:], ALU.mult)

            norm_stats(x_t, F32, lambda c, sl: x_t[c][:, sl], order=ob1)
            xh_t = [big.tile([128, L], mm_dt, tag=f"bufB{c}", name=f"bufB{c}")
                    for c in range(DC)]
            norm_apply(xh_t, lambda c, sl: x_t[c][:, sl], order=ob1)

            # ========================================== phase C: dw conv 3x3
            # column shifts via pre-shifted copies so every tap is a
            # row-contiguous 2D slice (f32r matmul alignment rules)
            hc_t = [big.tile([128, L], mm_dt, tag=f"bufA{c}", name=f"hc{c}")
                    for c in range(DC)]
            for c in range(DC):
                dg = load_w(diag_d[c], [128, 9 * 128], "dgw", bufs=1)
                for slab in ob1:
                    w0 = max(0, 7 * slab - 1)
                    w1 = min(NB, 7 * slab + 8)
                    nw = w1 - w0
                    win = xh_t[c][:, w0 * NB:w1 * NB].bitcast(F32)
                    xm = work.tile([128, 512], mm_dt, tag="xsh0", name="xm",
                                   bufs=1)
                    nc.vector.tensor_copy(xm[:, 1:nw * NB], win[:, 0:nw * NB - 1])
                    xm3 = xm[:, 0:nw * NB].rearrange("p (r cc) -> p r cc", cc=NB)
                    nc.vector.tensor_scalar(xm3[:, :, 0:1], xm3[:, :, 0:1],
                                            0.0, None, ALU.mult)
                    xp = work.tile([128, 512], mm_dt, tag="xsh1", name="xp",
                                   bufs=1)
                    nc.vector.tensor_copy(xp[:, 0:nw * NB - 1], win[:, 1:nw * NB])
                    xp3 = xp[:, 0:nw * NB].rearrange("p (r cc) -> p r cc", cc=NB)
                    nc.vector.tensor_scalar(xp3[:, :, NB - 1:NB],
                                            xp3[:, :, NB - 1:NB],
                                            0.0, None, ALU.mult)
                    cp = psumb.tile([128, NT], F32,
                                    tag=("hp_ps" if slab % 2 == 0 else "gp_ps"),
                                    name="conv_ps")
                    tap_order = [0, 3, 6, 2, 5, 8, 1, 4, 7]
                    for ti, tap in enumerate(tap_order):
                        dr, dcc, r0, r1, c0, c1 = _conv_tap_ranges(tap, slab)
                        nrow = r1 - r0
                        osl = slice((r0 - 7 * slab) * NB, (r1 - 7 * slab) * NB)
                        if dcc == 0:
                            rhs = xh_t[c][:, (r0 + dr) * NB:(r1 + dr) * NB]
                        elif dcc == -1:
                            rhs = xm[:, (r0 + dr - w0) * NB:(r1 + dr - w0) * NB]
                        else:
                            rhs = xp[:, (r0 + dr - w0) * NB:(r1 + dr - w0) * NB]
                        nc.tensor.matmul(
                            cp[:, osl], dg[:, tap * 128:(tap + 1) * 128], rhs,
                            start=(ti == 0), stop=(ti == 8))
                    nc.scalar.activation(
                        hc_t[c][:, slab * NT:(slab + 1) * NT], cp[:],
                        AF.Identity, bias=dwb_t[:, c:c + 1])

            # ========================================== phase G: bi-minGRU
            y_t = [big.tile([128, L], mm_dt, tag=f"bufB{c}", name=f"y{c}")
                   for c in range(DC)]
            carry = [work.tile([128, DIC], F32, tag=f"carry{g}",
                               name=f"carry{g}") for g in range(2)]

            for gi, g in enumerate((0, 1)):
                whg = [load_w((whg1_d if g == 0 else whg2_d)[k * 128:(k + 1) * 128, :],
                              [128, 2 * DI], f"whg{k}") for k in range(DC)]
                wout = [load_w((wout1_d if g == 0 else wout2_d)[k * 128:(k + 1) * 128, :],
                               [128, D], f"wout{k}", bufs=2) for k in range(DIC)]
                qorder = range(NQ) if g == 0 else range(NQ - 1, -1, -1)
                for qi, q in enumerate(qorder):
                    hs = [work.tile([128, QT], mm_dt, tag=f"hs{j}",
                                    name=f"hs{j}", bufs=1) for j in range(DIC)]
                    for j in range(DIC):
                        z = work.tile([128, QT], F32, tag="z", name="z", bufs=2)
                        s = work.tile([128, QT], F32, tag="s", name="s", bufs=2)
                        for nb2 in range(2):
                            nsl = slice(q * QT + nb2 * NT,
                                        q * QT + (nb2 + 1) * NT)
                            hsl = slice(nb2 * NT, (nb2 + 1) * NT)
                            hp = psumb.tile([128, NT], F32, tag="hp_ps",
                                            name="hp_ps")
                            gp = psumb.tile([128, NT], F32, tag="gp_ps",
                                            name="gp_ps")
                            for k in range(DC):
                                nc.tensor.matmul(
                                    hp[:], whg[k][:, j * 128:(j + 1) * 128],
                                    hc_t[k][:, nsl],
                                    start=(k == 0), stop=(k == DC - 1))
                            for k in range(DC):
                                nc.tensor.matmul(
                                    gp[:],
                                    whg[k][:, DI + j * 128:DI + (j + 1) * 128],
                                    hc_t[k][:, nsl],
                                    start=(k == 0), stop=(k == DC - 1))
                            nc.scalar.activation(z[:, hsl], gp[:], AF.Sigmoid)
                            nc.scalar.activation(s[:, hsl], hp[:], AF.Sigmoid)
                            # g = max(hidden+0.5, sigmoid(hidden)) in place
                            nc.vector.scalar_tensor_tensor(
                                s[:, hsl], hp[:], 0.5, s[:, hsl],
                                ALU.add, ALU.max)
                        bb = work.tile([128, QT], F32, tag="bb", name="bb",
                                       bufs=2)
                        nc.vector.tensor_tensor(bb[:], z[:], s[:], ALU.mult)
                        # a = 1 - z in place on z (after bb consumed z)
                        nc.scalar.activation(z[:], z[:], AF.Copy,
                                             bias=1.0, scale=-1.0)
                        init = 0.0 if qi == 0 else carry[g][:, j:j + 1]
                        if g == 0:
                            nc.vector.tensor_tensor_scan(
                                hs[j][:], z[:], bb[:], init, ALU.mult, ALU.add)
                            nc.gpsimd.tensor_copy(carry[g][:, j:j + 1],
                                                  hs[j][:, QT - 1:QT])
                        else:
                            rv = slice(None, None, -1)
                            nc.vector.tensor_tensor_scan(
                                hs[j][:, rv], z[:, rv], bb[:, rv], init,
                                ALU.mult, ALU.add)
                            nc.gpsimd.tensor_copy(carry[g][:, j:j + 1],
                                                  hs[j][:, 0:1])
                    for dc in range(DC):
                        for nb2 in range(2):
                            y_ps = psum.tile(
                                [128, NT], F32,
                                tag=("pC" if (dc * 2 + nb2) % 2 == 0 else "pD"),
                                name="y_ps")
                            for k in range(DIC):
                                nc.tensor.matmul(
                                    y_ps[:], wout[k][:, dc * 128:(dc + 1) * 128],
                                    hs[k][:, nb2 * NT:(nb2 + 1) * NT],
                                    start=(k == 0), stop=(k == DIC - 1))
                            ysl = slice(q * QT + nb2 * NT,
                                        q * QT + (nb2 + 1) * NT)
                            if gi == 0:
                                nc.scalar.activation(y_t[dc][:, ysl], y_ps[:],
                                                     AF.Copy)
                            else:
                                nc.vector.tensor_tensor(
                                    y_t[dc][:, ysl],
                                    y_t[dc][:, ysl].bitcast(F32), y_ps[:],
                                    ALU.add)
            # residual: y += xT (reload from DRAM)
            for blk in ob2:
                for c in range(DC):
                    sl = slice(blk * NT, (blk + 1) * NT)
                    xr = work.tile([128, NT], F32, tag="xr", name="xr",
                                   bufs=2)
                    nc.sync.dma_start(xr[:], xT_d[b, c * 128:(c + 1) * 128, sl])
                    nc.vector.tensor_tensor(y_t[c][:, sl],
                                            y_t[c][:, sl].bitcast(F32), xr[:],
                                            ALU.add)

            # ========================================== phase N2: layernorm2
            norm_stats(y_t, mm_dt,
                       lambda c, sl: y_t[c][:, sl].bitcast(F32), order=ob2)
            yh_t = [big.tile([128, L], mm_dt, tag=f"bufA{c}", name=f"yh{c}")
                    for c in range(DC)]
            norm_apply(yh_t, lambda c, sl: y_t[c][:, sl].bitcast(F32),
                       order=ob2)

            # ========================================== phase M: MLP (2-pass)
            p1w = [load_w(p1_d[k * 128:(k + 1) * 128, :], [128, MLP], f"whg{k}")
                   for k in range(DC)]
            p2w = [load_w(p2_d[k * 128:(k + 1) * 128, :], [128, D],
                          f"wout{k % 6}", bufs=2) for k in range(MLPC)]
            for blk in ob2:
                sl = slice(blk * NT, (blk + 1) * NT)
                ot0 = [work.tile([128, NT], F32, tag=["z", "s", "bb"][dc],
                                 name=f"ot0{dc}", bufs=2) for dc in range(DC)]
                for half in range(2):
                    qs = []
                    for mi in range(6):
                        mc = half * 6 + mi
                        q_ps = psum.tile([128, NT], F32,
                                         tag=("pA" if mi % 2 == 0 else "pB"),
                                         name="q_ps_m")
                        for k in range(DC):
                            nc.tensor.matmul(
                                q_ps[:], p1w[k][:, mc * 128:(mc + 1) * 128],
                                yh_t[k][:, sl],
                                start=(k == 0), stop=(k == DC - 1))
                        qt = work.tile([128, NT], mm_dt, tag=f"hs{mi}",
                                       name=f"q_sb{mi}", bufs=1)
                        nc.scalar.activation(qt[:], q_ps[:], AF.Gelu,
                                             bias=p1b_t[:, mc:mc + 1])
                        qs.append((mc, qt))
                    for dc in range(DC):
                        o_ps = psum.tile([128, NT], F32,
                                         tag=("pC" if dc % 2 == 0 else "pD"),
                                         name="o_ps")
                        for mi, (mc, qt) in enumerate(qs):
                            nc.tensor.matmul(
                                o_ps[:], p2w[mc][:, dc * 128:(dc + 1) * 128],
                                qt[:],
                                start=(mi == 0), stop=(mi == 5))
                        if half == 0:
                            nc.scalar.activation(ot0[dc][:], o_ps[:], AF.Copy)
                        else:
                            ot1 = work.tile([128, NT], F32, tag="sq",
                                            name="ot1", bufs=1)
                            nc.vector.scalar_tensor_tensor(
                                ot1[:], o_ps[:], p2b_t[:, dc:dc + 1],
                                y_t[dc][:, sl].bitcast(F32), ALU.add, ALU.add)
                            oo = work.tile([128, NT], F32, tag="t_ap",
                                           name="oo", bufs=3)
                            nc.vector.tensor_tensor(oo[:], ot0[dc][:], ot1[:],
                                                    ALU.add)
                            nc.sync.dma_start(
                                out_d[b, dc * 128:(dc + 1) * 128, sl], oo[:])

    return nc


# ---------------------------------------------------------------- host side
_NC_CACHE = {}


def _get_nc():
    key = str(MM_DT)
    if key not in _NC_CACHE:
        nc = build_kernel(MM_DT)
        _fix_multiwaits(nc)
        _NC_CACHE[key] = nc
    return _NC_CACHE[key]


def _prep_weights(inp):
    f = np.float32
    dw_w = np.asarray(inp["dw_w"], f)          # [D,1,3,3]
    norm_w = np.asarray(inp["norm_w"], f)
    norm_b = np.asarray(inp["norm_b"], f)
    dw_wf = dw_w[:, 0] * norm_w[:, None, None]     # [D,3,3]
    dw_bf = np.asarray(inp["dw_b"], f) + norm_b * dw_w[:, 0].sum(axis=(1, 2))
    p1_w = np.asarray(inp["p1_w"], f)
    p1f = p1_w * np.asarray(inp["norm2_w"], f)[:, None]
    p1bf = np.asarray(inp["p1_b"], f) + np.asarray(inp["norm2_b"], f) @ p1_w

    # conv diagonal weight blocks: [DC, 128, 9*128]
    diag = np.zeros((DC, 128, 9 * 128), f)
    ar = np.arange(128)
    for c in range(DC):
        for tap in range(9):
            dr, dcc = tap // 3, tap % 3
            diag[c, ar, tap * 128 + ar] = dw_wf[c * 128:(c + 1) * 128, dr, dcc]

    return dict(
        whg1=np.ascontiguousarray(inp["gru1_whg"], f),
        whg2=np.ascontiguousarray(inp["gru2_whg"], f),
        wout1=np.ascontiguousarray(inp["gru1_wout"], f),
        wout2=np.ascontiguousarray(inp["gru2_wout"], f),
        p1=np.ascontiguousarray(p1f, f),
        p2=np.ascontiguousarray(inp["p2_w"], f),
        diag=diag,
        dwb=np.ascontiguousarray(dw_bf.reshape(DC, 128).T, f),
        p1b=np.ascontiguousarray(p1bf.reshape(MLPC, 128).T, f),
        p2b=np.ascontiguousarray(np.asarray(inp["p2_b"], f).reshape(DC, 128).T, f),
    )


def kernel(**inputs):
    x = np.asarray(inputs["x"], np.float32)    # [16, L, D]
    n = x.shape[0]
    w = _prep_weights(inputs)
    nc = _get_nc()

    in_maps = []
    for core in range(NCORES):
        xb = x[core * B:(core + 1) * B]                   # [B, L, D]
        xT = np.ascontiguousarray(xb.transpose(0, 2, 1))  # [B, D, L]
        m = dict(w)
        m["xT"] = xT
        in_maps.append(m)

    res = run_bass_kernel_spmd(nc, in_maps, core_ids=list(range(NCORES)))
    outs = []
    for core in range(NCORES):
        oT = res.results[core]["outT"]                    # [B, D, L]
        outs.append(oT.transpose(0, 2, 1))                # [B, L, D]
    return np.ascontiguousarray(np.concatenate(outs, axis=0), np.float32)

